# revision 2
# baseline (speedup 1.0000x reference)
"""Trainium2 Bass kernel for the Dale's-law leaky RNN (nn_Net_20220706030448).

Data-parallel over batch across 8 NeuronCores (B=256 -> 32 per core).
State kept transposed [H-on-partitions, B-free]; recurrent weight held as
fp16 stationary tiles (fast weight load), fp32 PSUM accumulation and fp32
state. Input projection and output projection fused into the time loop in
4-step blocks.
"""

import os

import numpy as np

T, B, I, H, O = 1000, 256, 128, 512, 32
ALPHA = np.float32(20.0 / 100.0)
OMA = np.float32(1.0 - 20.0 / 100.0)
N_CORES = 8
BL = B // N_CORES  # 32 batch per core
TB = 4             # timesteps per block
NB = T // TB       # 250 blocks
HB = H // 128      # 4 h-blocks

LAST_RESULTS = None  # test harness reads exec_time_ns from here


def build_bass(t_steps=T):
    import concourse.bacc as bacc
    import concourse.mybir as mybir
    import concourse.tile as tile
    from concourse.masks import make_identity

    f32 = mybir.dt.float32
    f16 = mybir.dt.float16
    Relu = mybir.ActivationFunctionType.Relu
    Copy = mybir.ActivationFunctionType.Copy
    Ident = mybir.ActivationFunctionType.Identity

    nb = t_steps // TB
    assert t_steps % TB == 0

    nc = bacc.Bacc("TRN2", target_bir_lowering=False, debug=False,
                   num_devices=N_CORES)

    x_sl = nc.declare_dram_parameter("x_sl", [t_steps, BL, I], f32, isOutput=False)
    wrec_t = nc.declare_dram_parameter("wrec_t", [H, H], f16, isOutput=False)
    win_t = nc.declare_dram_parameter("win_t", [I, H], f16, isOutput=False)
    fcw_t = nc.declare_dram_parameter("fcw_t", [H, O], f16, isOutput=False)
    bvec = nc.declare_dram_parameter("bvec", [128, HB], f32, isOutput=False)
    fcb = nc.declare_dram_parameter("fcb", [O, 1], f32, isOutput=False)
    rnn_T = nc.declare_dram_parameter("rnn_T", [HB, 128, t_steps, BL], f32,
                                      isOutput=True)
    out_T = nc.declare_dram_parameter("out_T", [O, t_steps, BL], f32,
                                      isOutput=True)

    with tile.TileContext(nc) as tc:
        with (
            tc.tile_pool(name="const", bufs=1) as cpool,
            tc.tile_pool(name="xin", bufs=3) as xpool,
            tc.tile_pool(name="xt", bufs=2) as xtpool,
            tc.tile_pool(name="xd", bufs=2) as xdpool,
            tc.tile_pool(name="rb16", bufs=2) as rbpool,
            tc.tile_pool(name="rf32", bufs=2) as rfpool,
            tc.tile_pool(name="ot", bufs=2) as otpool,
            tc.tile_pool(name="ps_rec", bufs=3, space="PSUM") as ps_rec,
            tc.tile_pool(name="ps_xd", bufs=2, space="PSUM") as ps_xd,
            tc.tile_pool(name="ps_x", bufs=2, space="PSUM") as ps_x,
            tc.tile_pool(name="ps_out", bufs=1, space="PSUM") as ps_out,
        ):
            # ---- persistent tiles ----
            wrec_sb = cpool.tile([128, HB * H], f16, tag="wrec")
            for k in range(HB):
                nc.sync.dma_start(wrec_sb[:, k * H:(k + 1) * H],
                                  wrec_t[k * 128:(k + 1) * 128, :])
            win_sb = cpool.tile([128, H], f16, tag="win")
            nc.sync.dma_start(win_sb[:], win_t[:])
            fcw_sb = cpool.tile([128, HB * O], f16, tag="fcw")
            for k in range(HB):
                nc.sync.dma_start(fcw_sb[:, k * O:(k + 1) * O],
                                  fcw_t[k * 128:(k + 1) * 128, :])
            bvec_sb = cpool.tile([128, HB], f32, tag="bvec")
            nc.sync.dma_start(bvec_sb[:], bvec[:])
            fcb_sb = cpool.tile([O, 1], f32, tag="fcb")
            nc.sync.dma_start(fcb_sb[:], fcb[:])

            id_sb = cpool.tile([128, 128], f32, tag="ident")
            make_identity(nc, id_sb[:])

            zeros16 = cpool.tile([128, 128], f16, tag="zeros16")
            nc.vector.memset(zeros16[:], 0.0)
            s = cpool.tile([128, 128], f32, tag="state")
            nc.vector.memset(s[:], 0.0)
            pre = cpool.tile([128, 128], f32, tag="pre")

            prev_rb = None  # previous block's fp16 relu tile

            for ib in range(nb):
                t0 = ib * TB
                # -- load x block [(t,b)=128, i=128] and transpose to [i,(t,b)]
                xnat = xpool.tile([128, 128], f32, tag="xnat")
                nc.sync.dma_start(
                    xnat[:],
                    x_sl[t0:t0 + TB].rearrange("t b i -> (t b) i"))
                psx = ps_x.tile([128, 128], f32, tag="psx")
                nc.tensor.transpose(psx[:], xnat[:], id_sb[:])
                xT = xtpool.tile([128, 128], f16, tag="xT")
                nc.scalar.activation(xT[:], psx[:], Copy)

                # -- input drive for the block: psxd[:, m, (t b)] (2D matmul outs)
                psxd = ps_xd.tile([128, HB, TB * BL], f32, tag="psxd")
                for m in range(HB):
                    nc.tensor.matmul(
                        psxd[:, m, :],
                        win_sb[:, m * 128:(m + 1) * 128],
                        xT[:],
                        start=True, stop=True)
                # copy psum -> sbuf (t-major layout), adding per-h bias per m
                xd_sb = xdpool.tile([128, TB, HB, BL], f32, tag="xd")
                for m in range(HB):
                    nc.scalar.activation(
                        xd_sb[:, :, m, :],
                        psxd[:, m, :].rearrange("p (t b) -> p t b", t=TB),
                        Ident,
                        bias=bvec_sb[:, m:m + 1])

                rb = rbpool.tile([128, TB * 128], f16, tag="rb")
                rf = rfpool.tile([128, TB, HB, BL], f32, tag="rf")

                for t4 in range(TB):
                    t = t0 + t4
                    if t == 0:
                        prev = zeros16[:]
                    elif t4 == 0:
                        prev = prev_rb[:, (TB - 1) * 128:TB * 128]
                    else:
                        prev = rb[:, (t4 - 1) * 128:t4 * 128]

                    # off critical path: pre = OMA*s + xd_t
                    nc.vector.tensor_scalar_mul(pre[:], s[:], float(OMA))
                    nc.vector.tensor_add(pre[:], pre[:],
                                         xd_sb[:, t4, :, :].rearrange("p m b -> p (m b)"))

                    psr = ps_rec.tile([128, 128], f32, tag="psr")
                    for m in range(HB):
                        for k in range(HB):
                            nc.tensor.matmul(
                                psr[:, m * BL:(m + 1) * BL],
                                wrec_sb[:, k * H + m * 128:k * H + (m + 1) * 128],
                                prev[:, k * BL:(k + 1) * BL],
                                start=(k == 0), stop=(k == HB - 1))

                    # per-quarter: s_m = pre_m + psum_m ; relu -> fp16 (next rhs)
                    for m in range(HB):
                        q = slice(m * BL, (m + 1) * BL)
                        nc.vector.tensor_add(s[:, q], pre[:, q], psr[:, q])
                        nc.scalar.activation(
                            rb[:, t4 * 128 + m * 32:t4 * 128 + (m + 1) * 32],
                            s[:, q], Relu)
                    # fp32 relu for the rnn_activity output (off critical path)
                    nc.scalar.activation(
                        rf[:, t4, :, :].rearrange("p m b -> p (m b)"), s[:], Relu)

                # -- output projection for the block: out_T[o, t, b]
                rb_v = rb[:].rearrange("p (t k b) -> p t k b", t=TB, k=HB, b=BL)
                pso = ps_out.tile([O, TB * BL], f32, tag="pso")
                for k in range(HB):
                    nc.tensor.matmul(
                        pso[:],
                        fcw_sb[:, k * O:(k + 1) * O],
                        rb_v[:, :, k, :],
                        start=(k == 0), stop=(k == HB - 1))
                oT = otpool.tile([O, TB * BL], f32, tag="oT")
                nc.vector.tensor_scalar_add(oT[:], pso[:], fcb_sb[:, 0:1])
                nc.sync.dma_start(
                    out_T[:, t0:t0 + TB, :].rearrange("o t b -> o (t b)"), oT[:])

                # -- write fp32 relu block to rnn_T[k, p, t0:t0+4, :]
                for k in range(HB):
                    nc.sync.dma_start(rnn_T[k, :, t0:t0 + TB, :], rf[:, :, k, :])

                prev_rb = rb

    nc.compile()
    return nc


_BUILT = {}


def _get_nc(t_steps):
    if t_steps not in _BUILT:
        _BUILT[t_steps] = build_bass(t_steps)
    return _BUILT[t_steps]


def host_prep(w_in, b_in, w_h, b_h, dale, sparse, fc_w, fc_b):
    w_eff = np.maximum(w_h, 0.0) * dale[None, :] * sparse          # [H, H]
    wrec_t = (ALPHA * w_eff).T.astype(np.float16).copy()           # [H, H]
    win_t = (ALPHA * w_in).T.astype(np.float16).copy()             # [I, H]
    fcw_t = fc_w.T.astype(np.float16).copy()                       # [H, O]
    bv = (ALPHA * (b_in + b_h)).astype(np.float32)                 # [H]
    bvec = bv.reshape(HB, 128).T.copy()                            # [128, HB]
    fcb = fc_b.astype(np.float32).reshape(O, 1).copy()
    return wrec_t, win_t, fcw_t, bvec, fcb


def kernel(x, w_in, b_in, w_h, b_h, dale, sparse, fc_w, fc_b):
    from concourse.bass_utils import run_bass_kernel_spmd

    global LAST_RESULTS
    x = np.asarray(x, dtype=np.float32)
    wrec_t, win_t, fcw_t, bvec, fcb = host_prep(
        np.asarray(w_in, np.float32), np.asarray(b_in, np.float32),
        np.asarray(w_h, np.float32), np.asarray(b_h, np.float32),
        np.asarray(dale, np.float32), np.asarray(sparse, np.float32),
        np.asarray(fc_w, np.float32), np.asarray(fc_b, np.float32))

    t_steps = x.shape[0]
    nc = _get_nc(t_steps)

    in_maps = []
    for c in range(N_CORES):
        in_maps.append({
            "x_sl": np.ascontiguousarray(x[:, c * BL:(c + 1) * BL, :]),
            "wrec_t": wrec_t, "win_t": win_t, "fcw_t": fcw_t,
            "bvec": bvec, "fcb": fcb,
        })

    trace = bool(os.environ.get("BASS_TRACE"))
    res = run_bass_kernel_spmd(nc, in_maps, list(range(N_CORES)), trace=trace,
                               tmpdir=os.environ.get("BASS_TRACE_DIR"))
    LAST_RESULTS = res

    out = np.empty((t_steps, B, O), dtype=np.float32)
    rnn = np.empty((t_steps, B, H), dtype=np.float32)
    for c in range(N_CORES):
        r = res.results[c]
        bs = slice(c * BL, (c + 1) * BL)
        # rnn_T[k, p, t, b] -> [t, b, k*128+p]
        rnn[:, bs, :] = np.transpose(r["rnn_T"], (2, 3, 0, 1)).reshape(t_steps, BL, H)
        out[:, bs, :] = np.transpose(r["out_T"], (1, 2, 0))
    return out, rnn
